# revision 6
# baseline (speedup 1.0000x reference)
"""Trainium2 Bass kernel for the per-sample dynamic-depthwise-conv block.

Computation (per sample b):
    att  = sigmoid(lrelu(v @ ca_w1.T) @ ca_w2.T)            # [b, 64]
    kern = (lrelu(v @ k_w1.T) @ k_w2.T).reshape(b*64,1,3,3) # per-(b,c) 3x3
    y    = lrelu(depthwise3x3(x0 * att, kern))
    out  = conv1x1(y, conv_w) + conv_b

Strategy: data-parallel over batch across 8 cores (4 samples/core).  On each
core, samples are processed in 2 "pairs"; a pair's 2x64 channels fill the 128
SBUF partitions.  The attention gate is folded into the generated tap weights
(dw(att*x) == att*dw(x) per channel), so x0 streams straight from HBM as bf16
into a zero-padded [128, 130, 130] SBUF tile.  The 9 depthwise taps are split
across engines (5 PE / 4 DVE by default):
  - PE taps run as PSUM-accumulated matmuls with diagonal bf16 lhsT weights
    against shifted views of the padded tile;
  - DVE taps run as tensor_scalar at 4x mode + tensor_tensor add at 2x mode
    over 32-row blocks and are injected into the PSUM accumulation via
    identity matmuls.
Leaky-relu runs on the Scalar engine (Prelu, PSUM->SBUF, bf16); the final 1x1
conv is one matmul per 512-column chunk with a block-diagonal
[conv_w.T, conv_w.T] lhsT; conv bias rides the Scalar engine's Identity
activation during the PSUM->SBUF copy (one op per 1024 columns); outputs
leave as bf16 and are widened to fp32 on the host.  The tiny channel-
attention/kernel-generating MLPs run once per core on the PE/ACT engines.
"""

import sys

if "/opt/trn_rl_repo" not in sys.path:
    sys.path.append("/opt/trn_rl_repo")

import numpy as np
import ml_dtypes

B, C, H, W = 32, 64, 128, 128
KK = 3
RED = 8
N_CORES = 8
BPC = B // N_CORES          # samples per core (4)
PAIRS = BPC // 2            # sample pairs per core (2)
HP, WP = H + 2, W + 2       # zero-padded image dims (130)
ROWS_PER_CHUNK = 4          # output rows per matmul chunk -> N = 4*128 = 512
NCHUNK = H // ROWS_PER_CHUNK

_CACHE = {}

ALL_TAPS = [(di, dj) for di in range(KK) for dj in range(KK)]

CONFIGS = {
    # v2: GRP=2 psum groups, FD=1024 evacs on both ACT passes, hoisted
    # border memsets.
    "v2": dict(pe_taps=5, grp=2, big_psb=True, hoist=True),
    "v2p6": dict(pe_taps=6, grp=2, big_psb=True, hoist=True),
    "v2p4": dict(pe_taps=4, grp=2, big_psb=True, hoist=True),
    # v1: legacy structure (GRP=4, FD=512 bias evac, per-pair memsets)
    "p5": dict(pe_taps=5, grp=4, big_psb=False, hoist=False),
    "p6": dict(pe_taps=6, grp=4, big_psb=False, hoist=False),
    "p4": dict(pe_taps=4, grp=4, big_psb=False, hoist=False),
}


def _tap_split(pe_taps):
    # dj==1 taps are misaligned for DVE 2x/4x modes (bf16), keep them on PE.
    if pe_taps == 6:
        dve = [(0, 0), (0, 2), (1, 0)]
        pe = [(0, 1), (1, 1), (2, 1), (2, 0), (2, 2), (1, 2)]
    elif pe_taps == 4:
        dve = [(0, 0), (0, 2), (1, 0), (1, 2), (2, 0)]
        pe = [(0, 1), (1, 1), (2, 1), (2, 2)]
    else:
        dve = [(0, 0), (0, 2), (1, 0), (1, 2)]
        pe = [(0, 1), (1, 1), (2, 1), (2, 0), (2, 2)]
    return pe, dve


def _build(repeat=1, cfg="v2"):
    import concourse.bass as bass  # noqa: F401
    from concourse import bacc, tile, mybir

    cc = CONFIGS[cfg]
    f32 = mybir.dt.float32
    bf16 = mybir.dt.bfloat16
    AF = mybir.ActivationFunctionType

    nc = bacc.Bacc(None, target_bir_lowering=False, debug=False)

    x = nc.dram_tensor("x", [BPC, C, H, W], bf16, kind="ExternalInput")
    vt = nc.dram_tensor("vt", [C, BPC], f32, kind="ExternalInput")
    caw1t = nc.dram_tensor("caw1t", [C, RED], f32, kind="ExternalInput")
    caw2t = nc.dram_tensor("caw2t", [RED, C], f32, kind="ExternalInput")
    kw1t = nc.dram_tensor("kw1t", [C, C], f32, kind="ExternalInput")
    kw2t = nc.dram_tensor("kw2t", [C, C * KK * KK], f32, kind="ExternalInput")
    convt = nc.dram_tensor("convt", [128, 128], bf16, kind="ExternalInput")
    bcol = nc.dram_tensor("bcol", [128, 1], f32, kind="ExternalInput")
    eye = nc.dram_tensor("eye", [128, 128], f32, kind="ExternalInput")
    eyebf = nc.dram_tensor("eyebf", [128, 128], bf16, kind="ExternalInput")
    out = nc.dram_tensor("out", [BPC, C, H, W], bf16, kind="ExternalOutput")

    NK = C * KK * KK  # 576
    psa_bufs = 2 if cc["grp"] == 2 else 3

    with tile.TileContext(nc) as tc:
        with (
            tc.tile_pool(name="consts", bufs=1) as consts,
            tc.tile_pool(name="stage", bufs=1) as stage,
            tc.tile_pool(name="diags", bufs=1) as diags,
            tc.tile_pool(name="xin", bufs=2) as xin,
            tc.tile_pool(name="parts", bufs=2) as parts,
            tc.tile_pool(name="ys", bufs=5) as ys,
            tc.tile_pool(name="os", bufs=6) as osb,
            tc.tile_pool(name="psA", bufs=psa_bufs, space="PSUM") as psA,
            tc.tile_pool(name="psB", bufs=2, space="PSUM") as psB,
        ):
            # ---- constants into SBUF ----
            vt_sb = consts.tile([C, BPC], f32)
            caw1t_sb = consts.tile([C, RED], f32)
            caw2t_sb = consts.tile([RED, C], f32)
            kw1t_sb = consts.tile([C, C], f32)
            kw2t_sb = consts.tile([C, NK], f32)
            convt_sb = consts.tile([128, 128], bf16)
            bcol_sb = consts.tile([128, 1], f32)
            eye_sb = consts.tile([128, 128], f32)
            eyebf_sb = consts.tile([128, 128], bf16)
            for t, d in (
                (vt_sb, vt), (caw1t_sb, caw1t), (caw2t_sb, caw2t),
                (kw1t_sb, kw1t), (kw2t_sb, kw2t), (convt_sb, convt),
                (bcol_sb, bcol), (eye_sb, eye), (eyebf_sb, eyebf),
            ):
                nc.sync.dma_start(out=t[:], in_=d.ap())

            xts = None
            if cc["hoist"]:
                # Pre-zero the one-pixel border of both rotating x tiles
                # once; the interior is fully overwritten every pair and
                # the border stays zero across repeat iterations.
                xts = [xin.tile([128, HP, WP], bf16, tag="xt",
                                name=f"xt_pre{i}")
                       for i in range(2)]
                for xt in xts:
                    nc.vector.memset(xt[:, 0, :], 0.0)
                    nc.vector.memset(xt[:, HP - 1, :], 0.0)
                    nc.vector.memset(xt[:, 1:HP - 1, 0], 0.0)
                    nc.vector.memset(xt[:, 1:HP - 1, WP - 1], 0.0)

            import contextlib
            rep_ctx = (tc.For_i(0, repeat, 1) if repeat > 1
                       else contextlib.nullcontext())
            with rep_ctx:
                _body(nc, tc, mybir, AF, f32, bf16, cc,
                      consts, stage, diags, xin, parts, ys, osb,
                      psA, psB,
                      vt_sb, caw1t_sb, caw2t_sb, kw1t_sb, kw2t_sb,
                      convt_sb, bcol_sb, eye_sb, eyebf_sb, x, out)

    nc.compile()
    return nc


def _body(nc, tc, mybir, AF, f32, bf16, cc,
          consts, stage, diags, xin, parts, ys, osb, psA, psB,
          vt_sb, caw1t_sb, caw2t_sb, kw1t_sb, kw2t_sb,
          convt_sb, bcol_sb, eye_sb, eyebf_sb, x, out):
    NK = C * KK * KK
    PE_TAPS, DVE_TAPS = _tap_split(cc["pe_taps"])
    GRP = cc["grp"]

    # ---- tiny MLP stage: attention + generated kernels ----
    # h1T = lrelu(ca_w1 @ v.T)                       [8, BPC]
    ps_h1 = psA.tile([RED, BPC], f32, tag="pa", name="ps_h1")
    nc.tensor.matmul(ps_h1[:], lhsT=caw1t_sb[:], rhs=vt_sb[:],
                     start=True, stop=True)
    h1t_sb = stage.tile([RED, BPC], f32)
    nc.scalar.activation(h1t_sb[:], ps_h1[:], AF.Prelu, alpha=0.1)

    # attT = sigmoid(ca_w2 @ h1)                     [64, BPC]
    ps_att = psA.tile([C, BPC], f32, tag="pa", name="ps_att")
    nc.tensor.matmul(ps_att[:], lhsT=caw2t_sb[:], rhs=h1t_sb[:],
                     start=True, stop=True)
    att_sb = stage.tile([C, BPC], f32)
    nc.scalar.activation(att_sb[:], ps_att[:], AF.Sigmoid)

    # h2T = lrelu(k_w1 @ v.T)                        [64, BPC]
    ps_h2 = psA.tile([C, BPC], f32, tag="pa", name="ps_h2")
    nc.tensor.matmul(ps_h2[:], lhsT=kw1t_sb[:], rhs=vt_sb[:],
                     start=True, stop=True)
    h2t_sb = stage.tile([C, BPC], f32)
    nc.scalar.activation(h2t_sb[:], ps_h2[:], AF.Prelu, alpha=0.1)

    # kern = h2 @ k_w2.T                             [BPC, 576]
    ps_k = psA.tile([BPC, NK], f32, tag="pa", name="ps_k")
    nc.tensor.matmul(ps_k[:, 0:512], lhsT=h2t_sb[:],
                     rhs=kw2t_sb[:, 0:512], start=True, stop=True)
    nc.tensor.matmul(ps_k[:, 512:NK], lhsT=h2t_sb[:],
                     rhs=kw2t_sb[:, 512:NK], start=True, stop=True)
    kern_sb = stage.tile([BPC, NK], f32)
    nc.scalar.activation(kern_sb[:], ps_k[:], AF.Copy)

    # ---- gather per-pair tap scalars: p = s*64 + c on partitions ----
    dtap_sb = stage.tile([128, PAIRS, KK * KK], f32)
    attpp_sb = stage.tile([128, PAIRS], f32)
    for pr in range(PAIRS):
        for s in range(2):
            b = pr * 2 + s
            # src [1, 64, 9] and dest [64, 1, 9] match in flattened
            # element order (dma_start maps by flat AP order).
            src = kern_sb[b:b + 1, :].rearrange(
                "o (c t) -> o c t", c=C)
            nc.sync.dma_start(
                out=dtap_sb[C * s:C * (s + 1), pr:pr + 1, :], in_=src)
            nc.sync.dma_start(
                out=attpp_sb[C * s:C * (s + 1), pr:pr + 1],
                in_=att_sb[:, b:b + 1])

    # d[p, t] = att[p] * kern[p, t]; diag tiles = eye * d[:, t]
    diag = [{} for _ in range(PAIRS)]
    dcols = []
    for pr in range(PAIRS):
        d_pr = stage.tile([128, KK * KK], f32, tag=f"d{pr}")
        nc.vector.tensor_scalar_mul(
            d_pr[:], dtap_sb[:, pr, :], attpp_sb[:, pr:pr + 1])
        dcols.append(d_pr)
        for (di, dj) in PE_TAPS:
            t = di * KK + dj
            dg = diags.tile([128, 128], bf16, tag=f"diag{pr}_{t}")
            nc.vector.tensor_scalar_mul(
                dg[:], eye_sb[:], d_pr[:, t:t + 1])
            diag[pr][(di, dj)] = dg

    # ---- main loop ----
    xv = x.ap().rearrange("(pr s) c h w -> pr (s c) h w", pr=PAIRS)
    ov = out.ap().rearrange("(pr s) c h w -> pr (s c) h w", pr=PAIRS)

    NW = ROWS_PER_CHUNK * W  # 512
    for pr in range(PAIRS):
        xt = xin.tile([128, HP, WP], bf16, tag="xt")
        if not cc["hoist"]:
            nc.vector.memset(xt[:, 0, :], 0.0)
            nc.vector.memset(xt[:, HP - 1, :], 0.0)
            nc.vector.memset(xt[:, 1:HP - 1, 0], 0.0)
            nc.vector.memset(xt[:, 1:HP - 1, WP - 1], 0.0)
        # split the 4 MiB load across DMA queues
        nsplit = 8
        rstep = H // nsplit
        for k in range(nsplit):
            r0 = k * rstep
            nc.sync.dma_start(
                out=xt[:, 1 + r0:1 + r0 + rstep, 1:WP - 1],
                in_=xv[pr, :, r0:r0 + rstep, :])

        # Vector engine: even-aligned taps per 32-row block (bf16 2x/4x)
        BR = 32
        parts_of = {}
        for gb in range(0, NCHUNK, BR // ROWS_PER_CHUNK):
            r0 = (gb // (BR // ROWS_PER_CHUNK)) * BR
            part = parts.tile([128, BR, W], bf16, tag="part",
                              bufs=3, name=f"part{gb}")
            for g2 in range(gb, gb + BR // ROWS_PER_CHUNK):
                parts_of[g2] = (part, (g2 - gb) * ROWS_PER_CHUNK)
            for n, (di, dj) in enumerate(DVE_TAPS):
                t = di * KK + dj
                xin_v = xt[:, r0 + di:r0 + di + BR, dj:dj + W]
                if n == 0:
                    nc.vector.tensor_scalar_mul(
                        part[:], xin_v, dcols[pr][:, t:t + 1])
                else:
                    tmp = parts.tile([128, BR, W], bf16, tag="tmp",
                                     bufs=3, name=f"tmp{gb}_{n}")
                    nc.vector.tensor_scalar_mul(
                        tmp[:], xin_v, dcols[pr][:, t:t + 1])
                    nc.vector.tensor_add(part[:], part[:], tmp[:])

        if GRP == 2:
            for g in range(0, NCHUNK, 2):
                part, roff = parts_of[g]
                pa = psA.tile([128, 2 * NW], f32, tag="pa",
                              name=f"pa{pr}_{g}")
                for t, (di, dj) in enumerate(PE_TAPS):
                    for c in range(2):
                        i0 = (g + c) * ROWS_PER_CHUNK
                        nc.tensor.matmul(
                            pa[:, c * NW:c * NW + NW],
                            lhsT=diag[pr][(di, dj)][:],
                            rhs=xt[:, i0 + di:i0 + di + ROWS_PER_CHUNK,
                                   dj:dj + W],
                            start=(t == 0), stop=False,
                            skip_group_check=True)
                for c in range(2):
                    rc = roff + c * ROWS_PER_CHUNK
                    nc.tensor.matmul(
                        pa[:, c * NW:c * NW + NW],
                        lhsT=eyebf_sb[:],
                        rhs=part[:, rc:rc + ROWS_PER_CHUNK, :],
                        start=False, stop=True, skip_group_check=True)
                yt = ys.tile([128, 2 * NW], bf16, tag="yt")
                nc.scalar.activation(yt[:], pa[:], AF.Prelu, alpha=0.1)
                pb = psB.tile([128, 2 * NW], f32, tag="pb")
                for c2 in range(2):
                    nc.tensor.matmul(
                        pb[:, c2 * NW:c2 * NW + NW], lhsT=convt_sb[:],
                        rhs=yt[:, c2 * NW:c2 * NW + NW],
                        start=True, stop=True)
                ot = osb.tile([128, 2 * NW], bf16, tag="ot")
                nc.scalar.activation(ot[:], pb[:], AF.Identity,
                                     bias=bcol_sb[:, 0:1])
                j0 = g * ROWS_PER_CHUNK
                nc.sync.dma_start(
                    out=ov[pr, :, j0:j0 + 2 * ROWS_PER_CHUNK, :],
                    in_=ot[:].rearrange("p (r w) -> p r w",
                                        r=2 * ROWS_PER_CHUNK))
        else:
            for g in range(0, NCHUNK, GRP):
                part, roff = parts_of[g]
                pas = [psA.tile([128, 2 * NW], f32,
                                tag="pa", name=f"pa{g}_{h}")
                       for h in range(GRP // 2)]
                for t, (di, dj) in enumerate(PE_TAPS):
                    for c in range(GRP):
                        i0 = (g + c) * ROWS_PER_CHUNK
                        nc.tensor.matmul(
                            pas[c // 2][:, (c % 2) * NW:
                                        (c % 2) * NW + NW],
                            lhsT=diag[pr][(di, dj)][:],
                            rhs=xt[:, i0 + di:i0 + di + ROWS_PER_CHUNK,
                                   dj:dj + W],
                            start=(t == 0), stop=False,
                            skip_group_check=True)
                for c in range(GRP):
                    rc = roff + c * ROWS_PER_CHUNK
                    nc.tensor.matmul(
                        pas[c // 2][:, (c % 2) * NW:(c % 2) * NW + NW],
                        lhsT=eyebf_sb[:],
                        rhs=part[:, rc:rc + ROWS_PER_CHUNK, :],
                        start=False, stop=True, skip_group_check=True)
                for h in range(GRP // 2):
                    i0 = (g + 2 * h) * ROWS_PER_CHUNK
                    yt = ys.tile([128, 2 * NW], bf16, tag="yt")
                    nc.scalar.activation(yt[:], pas[h][:], AF.Prelu,
                                         alpha=0.1)
                    for c2 in range(2):
                        pb = psB.tile([128, NW], f32, tag="pb")
                        nc.tensor.matmul(
                            pb[:], lhsT=convt_sb[:],
                            rhs=yt[:, c2 * NW:c2 * NW + NW],
                            start=True, stop=True)
                        ot = osb.tile([128, NW], bf16, tag="ot")
                        nc.scalar.activation(ot[:], pb[:], AF.Identity,
                                             bias=bcol_sb[:, 0:1])
                        j0 = i0 + c2 * ROWS_PER_CHUNK
                        nc.sync.dma_start(
                            out=ov[pr, :, j0:j0 + ROWS_PER_CHUNK, :],
                            in_=ot[:].rearrange("p (r w) -> p r w",
                                                r=ROWS_PER_CHUNK))


def get_nc(repeat=1, cfg="v2"):
    key = ("nc", repeat, cfg)
    if key not in _CACHE:
        _CACHE[key] = _build(repeat, cfg)
    return _CACHE[key]


def make_in_maps(x0, v, ca_w1, ca_w2, k_w1, k_w2, conv_w, conv_b):
    bf = ml_dtypes.bfloat16
    caw1t = np.ascontiguousarray(ca_w1.T, dtype=np.float32)
    caw2t = np.ascontiguousarray(ca_w2.T, dtype=np.float32)
    kw1t = np.ascontiguousarray(k_w1.T, dtype=np.float32)
    kw2t = np.ascontiguousarray(k_w2.T, dtype=np.float32)
    convt = np.zeros((128, 128), dtype=bf)
    cwt = conv_w.T.astype(bf)
    convt[0:64, 0:64] = cwt
    convt[64:128, 64:128] = cwt
    bcol = np.tile(conv_b.astype(np.float32), 2)[:, None].copy()
    eye = np.eye(128, dtype=np.float32)
    eyebf = np.eye(128, dtype=bf)
    in_maps = []
    for k in range(N_CORES):
        sl = slice(k * BPC, (k + 1) * BPC)
        in_maps.append({
            "x": np.ascontiguousarray(x0[sl]).astype(bf),
            "vt": np.ascontiguousarray(v[sl].T, dtype=np.float32),
            "caw1t": caw1t, "caw2t": caw2t, "kw1t": kw1t, "kw2t": kw2t,
            "convt": convt, "bcol": bcol, "eye": eye, "eyebf": eyebf,
        })
    return in_maps


def kernel(x0, v, ca_w1, ca_w2, k_w1, k_w2, conv_w, conv_b):
    from concourse.bass_utils import run_bass_kernel_spmd

    nc = get_nc()
    in_maps = make_in_maps(x0, v, ca_w1, ca_w2, k_w1, k_w2, conv_w, conv_b)
    res = run_bass_kernel_spmd(nc, in_maps, list(range(N_CORES)))
    return np.concatenate([res.results[i]["out"] for i in range(N_CORES)],
                          axis=0).astype(np.float32)


# revision 14
# speedup vs baseline: 1.0424x; 1.0424x over previous
"""Trainium2 Bass kernel for the per-sample dynamic-depthwise-conv block.

Computation (per sample b):
    att  = sigmoid(lrelu(v @ ca_w1.T) @ ca_w2.T)            # [b, 64]
    kern = (lrelu(v @ k_w1.T) @ k_w2.T).reshape(b*64,1,3,3) # per-(b,c) 3x3
    y    = lrelu(depthwise3x3(x0 * att, kern))
    out  = conv1x1(y, conv_w) + conv_b

Strategy: data-parallel over batch across 8 cores (4 samples/core).  On each
core, samples are processed in 2 "pairs"; a pair's 2x64 channels fill the 128
SBUF partitions.  The attention gate is folded into the generated tap weights
(dw(att*x) == att*dw(x) per channel), so x0 streams straight from HBM as bf16
into a zero-padded [128, 130, 130] SBUF tile.  The 9 depthwise taps are split
across engines (5 PE / 4 DVE by default):
  - PE taps run as PSUM-accumulated matmuls with diagonal bf16 lhsT weights
    against shifted views of the padded tile;
  - DVE taps run as tensor_scalar at 4x mode + tensor_tensor add at 2x mode
    over 32-row blocks and are injected into the PSUM accumulation via
    identity matmuls.
Leaky-relu runs on the Scalar engine (Prelu, PSUM->SBUF, bf16); the final 1x1
conv is one matmul per 512-column chunk with a block-diagonal
[conv_w.T, conv_w.T] lhsT; conv bias rides the Scalar engine's Identity
activation during the PSUM->SBUF copy (one op per 1024 columns); outputs
leave as bf16 and are widened to fp32 on the host.  The tiny channel-
attention/kernel-generating MLPs run once per core on the PE/ACT engines.
"""

import sys

if "/opt/trn_rl_repo" not in sys.path:
    sys.path.append("/opt/trn_rl_repo")

import numpy as np
import ml_dtypes

B, C, H, W = 32, 64, 128, 128
KK = 3
RED = 8
N_CORES = 8
BPC = B // N_CORES          # samples per core (4)
PAIRS = BPC // 2            # sample pairs per core (2)
HP, WP = H + 2, W + 2       # zero-padded image dims (130)
ROWS_PER_CHUNK = 4          # output rows per matmul chunk -> N = 4*128 = 512
NCHUNK = H // ROWS_PER_CHUNK

_CACHE = {}

ALL_TAPS = [(di, dj) for di in range(KK) for dj in range(KK)]

WPP = 132  # host-padded image width (zero cols 0, 129..131); rows 0/129 zero

CONFIGS = {
    # v4: v3 + host-side zero-padded input in HBM -> single fully-contiguous
    # SBUF destination per x tile (4KB+ DMA descriptor runs instead of 256B,
    # which halves effective DMA bandwidth), no border memsets needed.
    "v4": dict(pe_taps=5, grp=4, big_psb=False, hoist=False, db=True,
               hbmpad=True),
    "v4p6": dict(pe_taps=6, grp=4, big_psb=False, hoist=False, db=True,
                 hbmpad=True),
    # v3: GRP=4 + hoisted border memsets + double-buffered MLP/diag stage
    # pools so iteration k+1's prep overlaps iteration k's main loop.
    "v3": dict(pe_taps=5, grp=4, big_psb=False, hoist=True, db=True),
    "v3p6": dict(pe_taps=6, grp=4, big_psb=False, hoist=True, db=True),
    # v2: GRP=2 psum groups, FD=1024 evacs on both ACT passes, hoisted
    # border memsets.
    "v2": dict(pe_taps=5, grp=2, big_psb=True, hoist=True),
    "v2p6": dict(pe_taps=6, grp=2, big_psb=True, hoist=True),
    "v2p4": dict(pe_taps=4, grp=2, big_psb=True, hoist=True),
    # v1: legacy structure (GRP=4, FD=512 bias evac, per-pair memsets)
    "p5": dict(pe_taps=5, grp=4, big_psb=False, hoist=False),
    "p6": dict(pe_taps=6, grp=4, big_psb=False, hoist=False),
    "p4": dict(pe_taps=4, grp=4, big_psb=False, hoist=False),
}


def _tap_split(pe_taps):
    # dj==1 taps are misaligned for DVE 2x/4x modes (bf16), keep them on PE.
    if pe_taps == 6:
        dve = [(0, 0), (0, 2), (1, 0)]
        pe = [(0, 1), (1, 1), (2, 1), (2, 0), (2, 2), (1, 2)]
    elif pe_taps == 4:
        dve = [(0, 0), (0, 2), (1, 0), (1, 2), (2, 0)]
        pe = [(0, 1), (1, 1), (2, 1), (2, 2)]
    else:
        dve = [(0, 0), (0, 2), (1, 0), (1, 2)]
        pe = [(0, 1), (1, 1), (2, 1), (2, 0), (2, 2)]
    return pe, dve


def _build(repeat=1, cfg="v2"):
    import concourse.bass as bass  # noqa: F401
    from concourse import bacc, tile, mybir

    cc = CONFIGS[cfg]
    f32 = mybir.dt.float32
    bf16 = mybir.dt.bfloat16
    AF = mybir.ActivationFunctionType

    nc = bacc.Bacc(None, target_bir_lowering=False, debug=False)

    if cc.get("hbmpad"):
        x = nc.dram_tensor("xp", [BPC, C, HP, WPP], bf16,
                           kind="ExternalInput")
    else:
        x = nc.dram_tensor("x", [BPC, C, H, W], bf16, kind="ExternalInput")
    vt = nc.dram_tensor("vt", [C, BPC], f32, kind="ExternalInput")
    caw1t = nc.dram_tensor("caw1t", [C, RED], f32, kind="ExternalInput")
    caw2t = nc.dram_tensor("caw2t", [RED, C], f32, kind="ExternalInput")
    kw1t = nc.dram_tensor("kw1t", [C, C], f32, kind="ExternalInput")
    kw2t = nc.dram_tensor("kw2t", [C, C * KK * KK], f32, kind="ExternalInput")
    convt = nc.dram_tensor("convt", [128, 128], bf16, kind="ExternalInput")
    bcol = nc.dram_tensor("bcol", [128, 1], f32, kind="ExternalInput")
    eye = nc.dram_tensor("eye", [128, 128], f32, kind="ExternalInput")
    eyebf = nc.dram_tensor("eyebf", [128, 128], bf16, kind="ExternalInput")
    out = nc.dram_tensor("out", [BPC, C, H, W], bf16, kind="ExternalOutput")

    NK = C * KK * KK  # 576
    psa_bufs = 2 if cc["grp"] == 2 else 3
    stage_bufs = 2 if cc.get("db") else 1

    with tile.TileContext(nc) as tc:
        with (
            tc.tile_pool(name="consts", bufs=1) as consts,
            tc.tile_pool(name="stage", bufs=stage_bufs) as stage,
            tc.tile_pool(name="diags", bufs=stage_bufs) as diags,
            tc.tile_pool(name="xin", bufs=2) as xin,
            tc.tile_pool(name="parts", bufs=2) as parts,
            tc.tile_pool(name="ys", bufs=5) as ys,
            tc.tile_pool(name="os", bufs=6) as osb,
            tc.tile_pool(name="psA", bufs=psa_bufs, space="PSUM") as psA,
            tc.tile_pool(name="psB", bufs=2, space="PSUM") as psB,
        ):
            # ---- constants into SBUF ----
            vt_sb = consts.tile([C, BPC], f32)
            caw1t_sb = consts.tile([C, RED], f32)
            caw2t_sb = consts.tile([RED, C], f32)
            kw1t_sb = consts.tile([C, C], f32)
            kw2t_sb = consts.tile([C, NK], f32)
            convt_sb = consts.tile([128, 128], bf16)
            bcol_sb = consts.tile([128, 1], f32)
            eye_sb = consts.tile([128, 128], f32)
            eyebf_sb = consts.tile([128, 128], bf16)
            for t, d in (
                (vt_sb, vt), (caw1t_sb, caw1t), (caw2t_sb, caw2t),
                (kw1t_sb, kw1t), (kw2t_sb, kw2t), (convt_sb, convt),
                (bcol_sb, bcol), (eye_sb, eye), (eyebf_sb, eyebf),
            ):
                nc.sync.dma_start(out=t[:], in_=d.ap())

            xts = None
            if cc["hoist"]:
                # Pre-zero the one-pixel border of both rotating x tiles
                # once; the interior is fully overwritten every pair and
                # the border stays zero across repeat iterations.
                xts = [xin.tile([128, HP, WP], bf16, tag="xt",
                                name=f"xt_pre{i}")
                       for i in range(2)]
                for xt in xts:
                    nc.vector.memset(xt[:, 0, :], 0.0)
                    nc.vector.memset(xt[:, HP - 1, :], 0.0)
                    nc.vector.memset(xt[:, 1:HP - 1, 0], 0.0)
                    nc.vector.memset(xt[:, 1:HP - 1, WP - 1], 0.0)

            import contextlib
            rep_ctx = (tc.For_i(0, repeat, 1) if repeat > 1
                       else contextlib.nullcontext())
            with rep_ctx:
                _body(nc, tc, mybir, AF, f32, bf16, cc,
                      consts, stage, diags, xin, parts, ys, osb,
                      psA, psB,
                      vt_sb, caw1t_sb, caw2t_sb, kw1t_sb, kw2t_sb,
                      convt_sb, bcol_sb, eye_sb, eyebf_sb, x, out)

    nc.compile()
    return nc


def _body(nc, tc, mybir, AF, f32, bf16, cc,
          consts, stage, diags, xin, parts, ys, osb, psA, psB,
          vt_sb, caw1t_sb, caw2t_sb, kw1t_sb, kw2t_sb,
          convt_sb, bcol_sb, eye_sb, eyebf_sb, x, out):
    NK = C * KK * KK
    PE_TAPS, DVE_TAPS = _tap_split(cc["pe_taps"])
    GRP = cc["grp"]

    # ---- tiny MLP stage: attention + generated kernels ----
    # h1T = lrelu(ca_w1 @ v.T)                       [8, BPC]
    ps_h1 = psA.tile([RED, BPC], f32, tag="pa", name="ps_h1")
    nc.tensor.matmul(ps_h1[:], lhsT=caw1t_sb[:], rhs=vt_sb[:],
                     start=True, stop=True)
    h1t_sb = stage.tile([RED, BPC], f32)
    nc.scalar.activation(h1t_sb[:], ps_h1[:], AF.Prelu, alpha=0.1)

    # attT = sigmoid(ca_w2 @ h1)                     [64, BPC]
    ps_att = psA.tile([C, BPC], f32, tag="pa", name="ps_att")
    nc.tensor.matmul(ps_att[:], lhsT=caw2t_sb[:], rhs=h1t_sb[:],
                     start=True, stop=True)
    att_sb = stage.tile([C, BPC], f32)
    nc.scalar.activation(att_sb[:], ps_att[:], AF.Sigmoid)

    # h2T = lrelu(k_w1 @ v.T)                        [64, BPC]
    ps_h2 = psA.tile([C, BPC], f32, tag="pa", name="ps_h2")
    nc.tensor.matmul(ps_h2[:], lhsT=kw1t_sb[:], rhs=vt_sb[:],
                     start=True, stop=True)
    h2t_sb = stage.tile([C, BPC], f32)
    nc.scalar.activation(h2t_sb[:], ps_h2[:], AF.Prelu, alpha=0.1)

    # kern = h2 @ k_w2.T                             [BPC, 576]
    ps_k = psA.tile([BPC, NK], f32, tag="pa", name="ps_k")
    nc.tensor.matmul(ps_k[:, 0:512], lhsT=h2t_sb[:],
                     rhs=kw2t_sb[:, 0:512], start=True, stop=True)
    nc.tensor.matmul(ps_k[:, 512:NK], lhsT=h2t_sb[:],
                     rhs=kw2t_sb[:, 512:NK], start=True, stop=True)
    kern_sb = stage.tile([BPC, NK], f32)
    nc.scalar.activation(kern_sb[:], ps_k[:], AF.Copy)

    # ---- gather per-pair tap scalars: p = s*64 + c on partitions ----
    dtap_sb = stage.tile([128, PAIRS, KK * KK], f32)
    attpp_sb = stage.tile([128, PAIRS], f32)
    for pr in range(PAIRS):
        for s in range(2):
            b = pr * 2 + s
            # src [1, 64, 9] and dest [64, 1, 9] match in flattened
            # element order (dma_start maps by flat AP order).
            src = kern_sb[b:b + 1, :].rearrange(
                "o (c t) -> o c t", c=C)
            nc.sync.dma_start(
                out=dtap_sb[C * s:C * (s + 1), pr:pr + 1, :], in_=src)
            nc.sync.dma_start(
                out=attpp_sb[C * s:C * (s + 1), pr:pr + 1],
                in_=att_sb[:, b:b + 1])

    # d[p, t] = att[p] * kern[p, t]; diag tiles = eye * d[:, t]
    diag = [{} for _ in range(PAIRS)]
    dcols = []
    for pr in range(PAIRS):
        d_pr = stage.tile([128, KK * KK], f32, tag=f"d{pr}")
        nc.vector.tensor_scalar_mul(
            d_pr[:], dtap_sb[:, pr, :], attpp_sb[:, pr:pr + 1])
        dcols.append(d_pr)
        for (di, dj) in PE_TAPS:
            t = di * KK + dj
            dg = diags.tile([128, 128], bf16, tag=f"diag{pr}_{t}")
            nc.vector.tensor_scalar_mul(
                dg[:], eye_sb[:], d_pr[:, t:t + 1])
            diag[pr][(di, dj)] = dg

    # ---- main loop ----
    xv = x.ap().rearrange("(pr s) c h w -> pr (s c) h w", pr=PAIRS)
    ov = out.ap().rearrange("(pr s) c h w -> pr (s c) h w", pr=PAIRS)

    NW = ROWS_PER_CHUNK * W  # 512
    for pr in range(PAIRS):
        if cc.get("hbmpad"):
            # input is zero-padded in HBM; each row-slice destination is
            # fully contiguous in SBUF (>= 4KB descriptor runs).
            xt = xin.tile([128, HP, WPP], bf16, tag="xt")
            nsplit = 8
            bounds = [round(HP * k / nsplit) for k in range(nsplit + 1)]
            for k in range(nsplit):
                a, b = bounds[k], bounds[k + 1]
                nc.sync.dma_start(
                    out=xt[:, a:b, :], in_=xv[pr, :, a:b, :])
        else:
            xt = xin.tile([128, HP, WP], bf16, tag="xt")
            if not cc["hoist"]:
                nc.vector.memset(xt[:, 0, :], 0.0)
                nc.vector.memset(xt[:, HP - 1, :], 0.0)
                nc.vector.memset(xt[:, 1:HP - 1, 0], 0.0)
                nc.vector.memset(xt[:, 1:HP - 1, WP - 1], 0.0)
            # split the 4 MiB load across DMA queues
            nsplit = 8
            rstep = H // nsplit
            for k in range(nsplit):
                r0 = k * rstep
                nc.sync.dma_start(
                    out=xt[:, 1 + r0:1 + r0 + rstep, 1:WP - 1],
                    in_=xv[pr, :, r0:r0 + rstep, :])

        # Vector engine: even-aligned taps per 32-row block (bf16 2x/4x)
        BR = 32
        parts_of = {}
        for gb in range(0, NCHUNK, BR // ROWS_PER_CHUNK):
            r0 = (gb // (BR // ROWS_PER_CHUNK)) * BR
            part = parts.tile([128, BR, W], bf16, tag="part",
                              bufs=3, name=f"part{gb}")
            for g2 in range(gb, gb + BR // ROWS_PER_CHUNK):
                parts_of[g2] = (part, (g2 - gb) * ROWS_PER_CHUNK)
            for n, (di, dj) in enumerate(DVE_TAPS):
                t = di * KK + dj
                xin_v = xt[:, r0 + di:r0 + di + BR, dj:dj + W]
                if n == 0:
                    nc.vector.tensor_scalar_mul(
                        part[:], xin_v, dcols[pr][:, t:t + 1])
                else:
                    tmp = parts.tile([128, BR, W], bf16, tag="tmp",
                                     bufs=3, name=f"tmp{gb}_{n}")
                    nc.vector.tensor_scalar_mul(
                        tmp[:], xin_v, dcols[pr][:, t:t + 1])
                    nc.vector.tensor_add(part[:], part[:], tmp[:])

        if GRP == 2:
            for g in range(0, NCHUNK, 2):
                part, roff = parts_of[g]
                pa = psA.tile([128, 2 * NW], f32, tag="pa",
                              name=f"pa{pr}_{g}")
                for t, (di, dj) in enumerate(PE_TAPS):
                    for c in range(2):
                        i0 = (g + c) * ROWS_PER_CHUNK
                        nc.tensor.matmul(
                            pa[:, c * NW:c * NW + NW],
                            lhsT=diag[pr][(di, dj)][:],
                            rhs=xt[:, i0 + di:i0 + di + ROWS_PER_CHUNK,
                                   dj:dj + W],
                            start=(t == 0), stop=False,
                            skip_group_check=True)
                for c in range(2):
                    rc = roff + c * ROWS_PER_CHUNK
                    nc.tensor.matmul(
                        pa[:, c * NW:c * NW + NW],
                        lhsT=eyebf_sb[:],
                        rhs=part[:, rc:rc + ROWS_PER_CHUNK, :],
                        start=False, stop=True, skip_group_check=True)
                yt = ys.tile([128, 2 * NW], bf16, tag="yt")
                nc.scalar.activation(yt[:], pa[:], AF.Prelu, alpha=0.1)
                pb = psB.tile([128, 2 * NW], f32, tag="pb")
                for c2 in range(2):
                    nc.tensor.matmul(
                        pb[:, c2 * NW:c2 * NW + NW], lhsT=convt_sb[:],
                        rhs=yt[:, c2 * NW:c2 * NW + NW],
                        start=True, stop=True)
                ot = osb.tile([128, 2 * NW], bf16, tag="ot")
                nc.scalar.activation(ot[:], pb[:], AF.Identity,
                                     bias=bcol_sb[:, 0:1])
                j0 = g * ROWS_PER_CHUNK
                nc.sync.dma_start(
                    out=ov[pr, :, j0:j0 + 2 * ROWS_PER_CHUNK, :],
                    in_=ot[:].rearrange("p (r w) -> p r w",
                                        r=2 * ROWS_PER_CHUNK))
        else:
            for g in range(0, NCHUNK, GRP):
                part, roff = parts_of[g]
                pas = [psA.tile([128, 2 * NW], f32,
                                tag="pa", name=f"pa{g}_{h}")
                       for h in range(GRP // 2)]
                for t, (di, dj) in enumerate(PE_TAPS):
                    for c in range(GRP):
                        i0 = (g + c) * ROWS_PER_CHUNK
                        nc.tensor.matmul(
                            pas[c // 2][:, (c % 2) * NW:
                                        (c % 2) * NW + NW],
                            lhsT=diag[pr][(di, dj)][:],
                            rhs=xt[:, i0 + di:i0 + di + ROWS_PER_CHUNK,
                                   dj:dj + W],
                            start=(t == 0), stop=False,
                            skip_group_check=True)
                for c in range(GRP):
                    rc = roff + c * ROWS_PER_CHUNK
                    nc.tensor.matmul(
                        pas[c // 2][:, (c % 2) * NW:(c % 2) * NW + NW],
                        lhsT=eyebf_sb[:],
                        rhs=part[:, rc:rc + ROWS_PER_CHUNK, :],
                        start=False, stop=True, skip_group_check=True)
                for h in range(GRP // 2):
                    i0 = (g + 2 * h) * ROWS_PER_CHUNK
                    yt = ys.tile([128, 2 * NW], bf16, tag="yt")
                    nc.scalar.activation(yt[:], pas[h][:], AF.Prelu,
                                         alpha=0.1)
                    for c2 in range(2):
                        pb = psB.tile([128, NW], f32, tag="pb")
                        nc.tensor.matmul(
                            pb[:], lhsT=convt_sb[:],
                            rhs=yt[:, c2 * NW:c2 * NW + NW],
                            start=True, stop=True)
                        ot = osb.tile([128, NW], bf16, tag="ot")
                        nc.scalar.activation(ot[:], pb[:], AF.Identity,
                                             bias=bcol_sb[:, 0:1])
                        j0 = i0 + c2 * ROWS_PER_CHUNK
                        nc.sync.dma_start(
                            out=ov[pr, :, j0:j0 + ROWS_PER_CHUNK, :],
                            in_=ot[:].rearrange("p (r w) -> p r w",
                                                r=ROWS_PER_CHUNK))


def get_nc(repeat=1, cfg="v4"):
    key = ("nc", repeat, cfg)
    if key not in _CACHE:
        _CACHE[key] = _build(repeat, cfg)
    return _CACHE[key]


def make_in_maps(x0, v, ca_w1, ca_w2, k_w1, k_w2, conv_w, conv_b):
    bf = ml_dtypes.bfloat16
    caw1t = np.ascontiguousarray(ca_w1.T, dtype=np.float32)
    caw2t = np.ascontiguousarray(ca_w2.T, dtype=np.float32)
    kw1t = np.ascontiguousarray(k_w1.T, dtype=np.float32)
    kw2t = np.ascontiguousarray(k_w2.T, dtype=np.float32)
    convt = np.zeros((128, 128), dtype=bf)
    cwt = conv_w.T.astype(bf)
    convt[0:64, 0:64] = cwt
    convt[64:128, 64:128] = cwt
    bcol = np.tile(conv_b.astype(np.float32), 2)[:, None].copy()
    eye = np.eye(128, dtype=np.float32)
    eyebf = np.eye(128, dtype=bf)
    x0bf = x0.astype(bf)
    xpad = np.zeros((B, C, HP, WPP), dtype=bf)
    xpad[:, :, 1:H + 1, 1:W + 1] = x0bf
    in_maps = []
    for k in range(N_CORES):
        sl = slice(k * BPC, (k + 1) * BPC)
        in_maps.append({
            "x": np.ascontiguousarray(x0bf[sl]),
            "xp": np.ascontiguousarray(xpad[sl]),
            "vt": np.ascontiguousarray(v[sl].T, dtype=np.float32),
            "caw1t": caw1t, "caw2t": caw2t, "kw1t": kw1t, "kw2t": kw2t,
            "convt": convt, "bcol": bcol, "eye": eye, "eyebf": eyebf,
        })
    return in_maps


def kernel(x0, v, ca_w1, ca_w2, k_w1, k_w2, conv_w, conv_b):
    from concourse.bass_utils import run_bass_kernel_spmd

    nc = get_nc()
    in_maps = make_in_maps(x0, v, ca_w1, ca_w2, k_w1, k_w2, conv_w, conv_b)
    res = run_bass_kernel_spmd(nc, in_maps, list(range(N_CORES)))
    return np.concatenate([res.results[i]["out"] for i in range(N_CORES)],
                          axis=0).astype(np.float32)


# revision 15
# speedup vs baseline: 1.0630x; 1.0197x over previous
"""Trainium2 Bass kernel for the per-sample dynamic-depthwise-conv block.

Computation (per sample b):
    att  = sigmoid(lrelu(v @ ca_w1.T) @ ca_w2.T)            # [b, 64]
    kern = (lrelu(v @ k_w1.T) @ k_w2.T).reshape(b*64,1,3,3) # per-(b,c) 3x3
    y    = lrelu(depthwise3x3(x0 * att, kern))
    out  = conv1x1(y, conv_w) + conv_b

Strategy: data-parallel over batch across 8 cores (4 samples/core).  On each
core, samples are processed in 2 "pairs"; a pair's 2x64 channels fill the 128
SBUF partitions.  The attention gate is folded into the generated tap weights
(dw(att*x) == att*dw(x) per channel), so x0 streams straight from HBM as bf16
into a zero-padded [128, 130, 130] SBUF tile.  The 9 depthwise taps are split
across engines (5 PE / 4 DVE by default):
  - PE taps run as PSUM-accumulated matmuls with diagonal bf16 lhsT weights
    against shifted views of the padded tile;
  - DVE taps run as tensor_scalar at 4x mode + tensor_tensor add at 2x mode
    over 32-row blocks and are injected into the PSUM accumulation via
    identity matmuls.
Leaky-relu runs on the Scalar engine (Prelu, PSUM->SBUF, bf16); the final 1x1
conv is one matmul per 512-column chunk with a block-diagonal
[conv_w.T, conv_w.T] lhsT; conv bias rides the Scalar engine's Identity
activation during the PSUM->SBUF copy (one op per 1024 columns); outputs
leave as bf16 and are widened to fp32 on the host.  The tiny channel-
attention/kernel-generating MLPs run once per core on the PE/ACT engines.
"""

import sys

if "/opt/trn_rl_repo" not in sys.path:
    sys.path.append("/opt/trn_rl_repo")

import numpy as np
import ml_dtypes

B, C, H, W = 32, 64, 128, 128
KK = 3
RED = 8
N_CORES = 8
BPC = B // N_CORES          # samples per core (4)
PAIRS = BPC // 2            # sample pairs per core (2)
HP, WP = H + 2, W + 2       # zero-padded image dims (130)
ROWS_PER_CHUNK = 4          # output rows per matmul chunk -> N = 4*128 = 512
NCHUNK = H // ROWS_PER_CHUNK

_CACHE = {}

ALL_TAPS = [(di, dj) for di in range(KK) for dj in range(KK)]

WPP = 132  # host-padded image width (zero cols 0, 129..131); rows 0/129 zero

CONFIGS = {
    # v5: v4 + software-pipelined prep: the MLP/gather/diag chain for the
    # next main-loop pass is issued ahead of the current pass (2-unrolled
    # repeat body with explicit slot tags), hiding its cross-engine latency.
    "v5": dict(pe_taps=5, grp=4, big_psb=False, hoist=False, db=False,
               hbmpad=True, skew=True),
    "v5p6": dict(pe_taps=6, grp=4, big_psb=False, hoist=False, db=False,
                 hbmpad=True, skew=True),
    # v4: v3 + host-side zero-padded input in HBM -> single fully-contiguous
    # SBUF destination per x tile (4KB+ DMA descriptor runs instead of 256B,
    # which halves effective DMA bandwidth), no border memsets needed.
    "v4": dict(pe_taps=5, grp=4, big_psb=False, hoist=False, db=True,
               hbmpad=True),
    "v4p6": dict(pe_taps=6, grp=4, big_psb=False, hoist=False, db=True,
                 hbmpad=True),
    # v3: GRP=4 + hoisted border memsets + double-buffered MLP/diag stage
    # pools so iteration k+1's prep overlaps iteration k's main loop.
    "v3": dict(pe_taps=5, grp=4, big_psb=False, hoist=True, db=True),
    "v3p6": dict(pe_taps=6, grp=4, big_psb=False, hoist=True, db=True),
    # v2: GRP=2 psum groups, FD=1024 evacs on both ACT passes, hoisted
    # border memsets.
    "v2": dict(pe_taps=5, grp=2, big_psb=True, hoist=True),
    "v2p6": dict(pe_taps=6, grp=2, big_psb=True, hoist=True),
    "v2p4": dict(pe_taps=4, grp=2, big_psb=True, hoist=True),
    # v1: legacy structure (GRP=4, FD=512 bias evac, per-pair memsets)
    "p5": dict(pe_taps=5, grp=4, big_psb=False, hoist=False),
    "p6": dict(pe_taps=6, grp=4, big_psb=False, hoist=False),
    "p4": dict(pe_taps=4, grp=4, big_psb=False, hoist=False),
}


def _tap_split(pe_taps):
    # dj==1 taps are misaligned for DVE 2x/4x modes (bf16), keep them on PE.
    if pe_taps == 6:
        dve = [(0, 0), (0, 2), (1, 0)]
        pe = [(0, 1), (1, 1), (2, 1), (2, 0), (2, 2), (1, 2)]
    elif pe_taps == 4:
        dve = [(0, 0), (0, 2), (1, 0), (1, 2), (2, 0)]
        pe = [(0, 1), (1, 1), (2, 1), (2, 2)]
    else:
        dve = [(0, 0), (0, 2), (1, 0), (1, 2)]
        pe = [(0, 1), (1, 1), (2, 1), (2, 0), (2, 2)]
    return pe, dve


def _build(repeat=1, cfg="v2"):
    import concourse.bass as bass  # noqa: F401
    from concourse import bacc, tile, mybir

    cc = CONFIGS[cfg]
    f32 = mybir.dt.float32
    bf16 = mybir.dt.bfloat16
    AF = mybir.ActivationFunctionType

    nc = bacc.Bacc(None, target_bir_lowering=False, debug=False)

    if cc.get("hbmpad"):
        x = nc.dram_tensor("xp", [BPC, C, HP, WPP], bf16,
                           kind="ExternalInput")
    else:
        x = nc.dram_tensor("x", [BPC, C, H, W], bf16, kind="ExternalInput")
    vt = nc.dram_tensor("vt", [C, BPC], f32, kind="ExternalInput")
    caw1t = nc.dram_tensor("caw1t", [C, RED], f32, kind="ExternalInput")
    caw2t = nc.dram_tensor("caw2t", [RED, C], f32, kind="ExternalInput")
    kw1t = nc.dram_tensor("kw1t", [C, C], f32, kind="ExternalInput")
    kw2t = nc.dram_tensor("kw2t", [C, C * KK * KK], f32, kind="ExternalInput")
    convt = nc.dram_tensor("convt", [128, 128], bf16, kind="ExternalInput")
    bcol = nc.dram_tensor("bcol", [128, 1], f32, kind="ExternalInput")
    eye = nc.dram_tensor("eye", [128, 128], f32, kind="ExternalInput")
    eyebf = nc.dram_tensor("eyebf", [128, 128], bf16, kind="ExternalInput")
    out = nc.dram_tensor("out", [BPC, C, H, W], bf16, kind="ExternalOutput")

    NK = C * KK * KK  # 576
    psa_bufs = 2 if cc["grp"] == 2 else 3
    stage_bufs = 2 if cc.get("db") else 1

    with tile.TileContext(nc) as tc:
        with (
            tc.tile_pool(name="consts", bufs=1) as consts,
            tc.tile_pool(name="stage", bufs=stage_bufs) as stage,
            tc.tile_pool(name="diags", bufs=stage_bufs) as diags,
            tc.tile_pool(name="xin", bufs=2) as xin,
            tc.tile_pool(name="parts", bufs=2) as parts,
            tc.tile_pool(name="ys", bufs=5) as ys,
            tc.tile_pool(name="os", bufs=6) as osb,
            tc.tile_pool(name="psA", bufs=psa_bufs, space="PSUM") as psA,
            tc.tile_pool(name="psB", bufs=2, space="PSUM") as psB,
        ):
            # ---- constants into SBUF ----
            vt_sb = consts.tile([C, BPC], f32)
            caw1t_sb = consts.tile([C, RED], f32)
            caw2t_sb = consts.tile([RED, C], f32)
            kw1t_sb = consts.tile([C, C], f32)
            kw2t_sb = consts.tile([C, NK], f32)
            convt_sb = consts.tile([128, 128], bf16)
            bcol_sb = consts.tile([128, 1], f32)
            eye_sb = consts.tile([128, 128], f32)
            eyebf_sb = consts.tile([128, 128], bf16)
            for t, d in (
                (vt_sb, vt), (caw1t_sb, caw1t), (caw2t_sb, caw2t),
                (kw1t_sb, kw1t), (kw2t_sb, kw2t), (convt_sb, convt),
                (bcol_sb, bcol), (eye_sb, eye), (eyebf_sb, eyebf),
            ):
                nc.sync.dma_start(out=t[:], in_=d.ap())

            xts = None
            if cc["hoist"]:
                # Pre-zero the one-pixel border of both rotating x tiles
                # once; the interior is fully overwritten every pair and
                # the border stays zero across repeat iterations.
                xts = [xin.tile([128, HP, WP], bf16, tag="xt",
                                name=f"xt_pre{i}")
                       for i in range(2)]
                for xt in xts:
                    nc.vector.memset(xt[:, 0, :], 0.0)
                    nc.vector.memset(xt[:, HP - 1, :], 0.0)
                    nc.vector.memset(xt[:, 1:HP - 1, 0], 0.0)
                    nc.vector.memset(xt[:, 1:HP - 1, WP - 1], 0.0)

            import contextlib
            rep_ctx = (tc.For_i(0, repeat, 1) if repeat > 1
                       else contextlib.nullcontext())
            with rep_ctx:
                _body(nc, tc, mybir, AF, f32, bf16, cc,
                      consts, stage, diags, xin, parts, ys, osb,
                      psA, psB,
                      vt_sb, caw1t_sb, caw2t_sb, kw1t_sb, kw2t_sb,
                      convt_sb, bcol_sb, eye_sb, eyebf_sb, x, out)

    nc.compile()
    return nc


def _body(nc, tc, mybir, AF, f32, bf16, cc,
          consts, stage, diags, xin, parts, ys, osb, psA, psB,
          vt_sb, caw1t_sb, caw2t_sb, kw1t_sb, kw2t_sb,
          convt_sb, bcol_sb, eye_sb, eyebf_sb, x, out):
    NK = C * KK * KK
    PE_TAPS, DVE_TAPS = _tap_split(cc["pe_taps"])
    GRP = cc["grp"]

    # ---- tiny MLP stage: attention + generated kernels ----
    # h1T = lrelu(ca_w1 @ v.T)                       [8, BPC]
    ps_h1 = psA.tile([RED, BPC], f32, tag="pa", name="ps_h1")
    nc.tensor.matmul(ps_h1[:], lhsT=caw1t_sb[:], rhs=vt_sb[:],
                     start=True, stop=True)
    h1t_sb = stage.tile([RED, BPC], f32)
    nc.scalar.activation(h1t_sb[:], ps_h1[:], AF.Prelu, alpha=0.1)

    # attT = sigmoid(ca_w2 @ h1)                     [64, BPC]
    ps_att = psA.tile([C, BPC], f32, tag="pa", name="ps_att")
    nc.tensor.matmul(ps_att[:], lhsT=caw2t_sb[:], rhs=h1t_sb[:],
                     start=True, stop=True)
    att_sb = stage.tile([C, BPC], f32)
    nc.scalar.activation(att_sb[:], ps_att[:], AF.Sigmoid)

    # h2T = lrelu(k_w1 @ v.T)                        [64, BPC]
    ps_h2 = psA.tile([C, BPC], f32, tag="pa", name="ps_h2")
    nc.tensor.matmul(ps_h2[:], lhsT=kw1t_sb[:], rhs=vt_sb[:],
                     start=True, stop=True)
    h2t_sb = stage.tile([C, BPC], f32)
    nc.scalar.activation(h2t_sb[:], ps_h2[:], AF.Prelu, alpha=0.1)

    # kern = h2 @ k_w2.T                             [BPC, 576]
    ps_k = psA.tile([BPC, NK], f32, tag="pa", name="ps_k")
    nc.tensor.matmul(ps_k[:, 0:512], lhsT=h2t_sb[:],
                     rhs=kw2t_sb[:, 0:512], start=True, stop=True)
    nc.tensor.matmul(ps_k[:, 512:NK], lhsT=h2t_sb[:],
                     rhs=kw2t_sb[:, 512:NK], start=True, stop=True)
    kern_sb = stage.tile([BPC, NK], f32)
    nc.scalar.activation(kern_sb[:], ps_k[:], AF.Copy)

    # ---- gather per-pair tap scalars: p = s*64 + c on partitions ----
    dtap_sb = stage.tile([128, PAIRS, KK * KK], f32)
    attpp_sb = stage.tile([128, PAIRS], f32)
    for pr in range(PAIRS):
        for s in range(2):
            b = pr * 2 + s
            # src [1, 64, 9] and dest [64, 1, 9] match in flattened
            # element order (dma_start maps by flat AP order).
            src = kern_sb[b:b + 1, :].rearrange(
                "o (c t) -> o c t", c=C)
            nc.sync.dma_start(
                out=dtap_sb[C * s:C * (s + 1), pr:pr + 1, :], in_=src)
            nc.sync.dma_start(
                out=attpp_sb[C * s:C * (s + 1), pr:pr + 1],
                in_=att_sb[:, b:b + 1])

    # d[p, t] = att[p] * kern[p, t]; diag tiles = eye * d[:, t]
    diag = [{} for _ in range(PAIRS)]
    dcols = []
    for pr in range(PAIRS):
        d_pr = stage.tile([128, KK * KK], f32, tag=f"d{pr}")
        nc.vector.tensor_scalar_mul(
            d_pr[:], dtap_sb[:, pr, :], attpp_sb[:, pr:pr + 1])
        dcols.append(d_pr)
        for (di, dj) in PE_TAPS:
            t = di * KK + dj
            dg = diags.tile([128, 128], bf16, tag=f"diag{pr}_{t}")
            nc.vector.tensor_scalar_mul(
                dg[:], eye_sb[:], d_pr[:, t:t + 1])
            diag[pr][(di, dj)] = dg

    # ---- main loop ----
    xv = x.ap().rearrange("(pr s) c h w -> pr (s c) h w", pr=PAIRS)
    ov = out.ap().rearrange("(pr s) c h w -> pr (s c) h w", pr=PAIRS)

    NW = ROWS_PER_CHUNK * W  # 512
    for pr in range(PAIRS):
        if cc.get("hbmpad"):
            # input is zero-padded in HBM; each row-slice destination is
            # fully contiguous in SBUF (>= 4KB descriptor runs).
            xt = xin.tile([128, HP, WPP], bf16, tag="xt")
            nsplit = 8
            bounds = [round(HP * k / nsplit) for k in range(nsplit + 1)]
            for k in range(nsplit):
                a, b = bounds[k], bounds[k + 1]
                nc.sync.dma_start(
                    out=xt[:, a:b, :], in_=xv[pr, :, a:b, :])
        else:
            xt = xin.tile([128, HP, WP], bf16, tag="xt")
            if not cc["hoist"]:
                nc.vector.memset(xt[:, 0, :], 0.0)
                nc.vector.memset(xt[:, HP - 1, :], 0.0)
                nc.vector.memset(xt[:, 1:HP - 1, 0], 0.0)
                nc.vector.memset(xt[:, 1:HP - 1, WP - 1], 0.0)
            # split the 4 MiB load across DMA queues
            nsplit = 8
            rstep = H // nsplit
            for k in range(nsplit):
                r0 = k * rstep
                nc.sync.dma_start(
                    out=xt[:, 1 + r0:1 + r0 + rstep, 1:WP - 1],
                    in_=xv[pr, :, r0:r0 + rstep, :])

        # Vector engine: even-aligned taps per 32-row block (bf16 2x/4x)
        BR = 32
        parts_of = {}
        for gb in range(0, NCHUNK, BR // ROWS_PER_CHUNK):
            r0 = (gb // (BR // ROWS_PER_CHUNK)) * BR
            part = parts.tile([128, BR, W], bf16, tag="part",
                              bufs=3, name=f"part{gb}")
            for g2 in range(gb, gb + BR // ROWS_PER_CHUNK):
                parts_of[g2] = (part, (g2 - gb) * ROWS_PER_CHUNK)
            for n, (di, dj) in enumerate(DVE_TAPS):
                t = di * KK + dj
                xin_v = xt[:, r0 + di:r0 + di + BR, dj:dj + W]
                if n == 0:
                    nc.vector.tensor_scalar_mul(
                        part[:], xin_v, dcols[pr][:, t:t + 1])
                else:
                    tmp = parts.tile([128, BR, W], bf16, tag="tmp",
                                     bufs=3, name=f"tmp{gb}_{n}")
                    nc.vector.tensor_scalar_mul(
                        tmp[:], xin_v, dcols[pr][:, t:t + 1])
                    nc.vector.tensor_add(part[:], part[:], tmp[:])

        if GRP == 2:
            for g in range(0, NCHUNK, 2):
                part, roff = parts_of[g]
                pa = psA.tile([128, 2 * NW], f32, tag="pa",
                              name=f"pa{pr}_{g}")
                for t, (di, dj) in enumerate(PE_TAPS):
                    for c in range(2):
                        i0 = (g + c) * ROWS_PER_CHUNK
                        nc.tensor.matmul(
                            pa[:, c * NW:c * NW + NW],
                            lhsT=diag[pr][(di, dj)][:],
                            rhs=xt[:, i0 + di:i0 + di + ROWS_PER_CHUNK,
                                   dj:dj + W],
                            start=(t == 0), stop=False,
                            skip_group_check=True)
                for c in range(2):
                    rc = roff + c * ROWS_PER_CHUNK
                    nc.tensor.matmul(
                        pa[:, c * NW:c * NW + NW],
                        lhsT=eyebf_sb[:],
                        rhs=part[:, rc:rc + ROWS_PER_CHUNK, :],
                        start=False, stop=True, skip_group_check=True)
                yt = ys.tile([128, 2 * NW], bf16, tag="yt")
                nc.scalar.activation(yt[:], pa[:], AF.Prelu, alpha=0.1)
                pb = psB.tile([128, 2 * NW], f32, tag="pb")
                for c2 in range(2):
                    nc.tensor.matmul(
                        pb[:, c2 * NW:c2 * NW + NW], lhsT=convt_sb[:],
                        rhs=yt[:, c2 * NW:c2 * NW + NW],
                        start=True, stop=True)
                ot = osb.tile([128, 2 * NW], bf16, tag="ot")
                nc.scalar.activation(ot[:], pb[:], AF.Identity,
                                     bias=bcol_sb[:, 0:1])
                j0 = g * ROWS_PER_CHUNK
                nc.sync.dma_start(
                    out=ov[pr, :, j0:j0 + 2 * ROWS_PER_CHUNK, :],
                    in_=ot[:].rearrange("p (r w) -> p r w",
                                        r=2 * ROWS_PER_CHUNK))
        else:
            for g in range(0, NCHUNK, GRP):
                part, roff = parts_of[g]
                pas = [psA.tile([128, 2 * NW], f32,
                                tag="pa", name=f"pa{g}_{h}")
                       for h in range(GRP // 2)]
                for t, (di, dj) in enumerate(PE_TAPS):
                    for c in range(GRP):
                        i0 = (g + c) * ROWS_PER_CHUNK
                        nc.tensor.matmul(
                            pas[c // 2][:, (c % 2) * NW:
                                        (c % 2) * NW + NW],
                            lhsT=diag[pr][(di, dj)][:],
                            rhs=xt[:, i0 + di:i0 + di + ROWS_PER_CHUNK,
                                   dj:dj + W],
                            start=(t == 0), stop=False,
                            skip_group_check=True)
                for c in range(GRP):
                    rc = roff + c * ROWS_PER_CHUNK
                    nc.tensor.matmul(
                        pas[c // 2][:, (c % 2) * NW:(c % 2) * NW + NW],
                        lhsT=eyebf_sb[:],
                        rhs=part[:, rc:rc + ROWS_PER_CHUNK, :],
                        start=False, stop=True, skip_group_check=True)
                for h in range(GRP // 2):
                    i0 = (g + 2 * h) * ROWS_PER_CHUNK
                    yt = ys.tile([128, 2 * NW], bf16, tag="yt")
                    nc.scalar.activation(yt[:], pas[h][:], AF.Prelu,
                                         alpha=0.1)
                    for c2 in range(2):
                        pb = psB.tile([128, NW], f32, tag="pb")
                        nc.tensor.matmul(
                            pb[:], lhsT=convt_sb[:],
                            rhs=yt[:, c2 * NW:c2 * NW + NW],
                            start=True, stop=True)
                        ot = osb.tile([128, NW], bf16, tag="ot")
                        nc.scalar.activation(ot[:], pb[:], AF.Identity,
                                             bias=bcol_sb[:, 0:1])
                        j0 = i0 + c2 * ROWS_PER_CHUNK
                        nc.sync.dma_start(
                            out=ov[pr, :, j0:j0 + ROWS_PER_CHUNK, :],
                            in_=ot[:].rearrange("p (r w) -> p r w",
                                                r=ROWS_PER_CHUNK))


def get_nc(repeat=1, cfg="v4"):
    key = ("nc", repeat, cfg)
    if key not in _CACHE:
        _CACHE[key] = _build(repeat, cfg)
    return _CACHE[key]


def make_in_maps(x0, v, ca_w1, ca_w2, k_w1, k_w2, conv_w, conv_b):
    bf = ml_dtypes.bfloat16
    caw1t = np.ascontiguousarray(ca_w1.T, dtype=np.float32)
    caw2t = np.ascontiguousarray(ca_w2.T, dtype=np.float32)
    kw1t = np.ascontiguousarray(k_w1.T, dtype=np.float32)
    kw2t = np.ascontiguousarray(k_w2.T, dtype=np.float32)
    convt = np.zeros((128, 128), dtype=bf)
    cwt = conv_w.T.astype(bf)
    convt[0:64, 0:64] = cwt
    convt[64:128, 64:128] = cwt
    bcol = np.tile(conv_b.astype(np.float32), 2)[:, None].copy()
    eye = np.eye(128, dtype=np.float32)
    eyebf = np.eye(128, dtype=bf)
    x0bf = x0.astype(bf)
    xpad = np.zeros((B, C, HP, WPP), dtype=bf)
    xpad[:, :, 1:H + 1, 1:W + 1] = x0bf
    in_maps = []
    for k in range(N_CORES):
        sl = slice(k * BPC, (k + 1) * BPC)
        in_maps.append({
            "x": np.ascontiguousarray(x0bf[sl]),
            "xp": np.ascontiguousarray(xpad[sl]),
            "vt": np.ascontiguousarray(v[sl].T, dtype=np.float32),
            "caw1t": caw1t, "caw2t": caw2t, "kw1t": kw1t, "kw2t": kw2t,
            "convt": convt, "bcol": bcol, "eye": eye, "eyebf": eyebf,
        })
    return in_maps


def kernel(x0, v, ca_w1, ca_w2, k_w1, k_w2, conv_w, conv_b):
    from concourse.bass_utils import run_bass_kernel_spmd

    nc = get_nc()
    in_maps = make_in_maps(x0, v, ca_w1, ca_w2, k_w1, k_w2, conv_w, conv_b)
    res = run_bass_kernel_spmd(nc, in_maps, list(range(N_CORES)))
    return np.concatenate([res.results[i]["out"] for i in range(N_CORES)],
                          axis=0).astype(np.float32)


# revision 21
# speedup vs baseline: 1.1236x; 1.0570x over previous
"""Trainium2 Bass kernel for the per-sample dynamic-depthwise-conv block.

Computation (per sample b):
    att  = sigmoid(lrelu(v @ ca_w1.T) @ ca_w2.T)            # [b, 64]
    kern = (lrelu(v @ k_w1.T) @ k_w2.T).reshape(b*64,1,3,3) # per-(b,c) 3x3
    y    = lrelu(depthwise3x3(x0 * att, kern))
    out  = conv1x1(y, conv_w) + conv_b

Strategy: data-parallel over batch across 8 cores (4 samples/core).  On each
core, samples are processed in 2 "pairs"; a pair's 2x64 channels fill the 128
SBUF partitions.  The attention gate is folded into the generated tap weights
(dw(att*x) == att*dw(x) per channel), so x0 streams straight from HBM as bf16
into a zero-padded [128, 130, 130] SBUF tile.  The 9 depthwise taps are split
across engines (5 PE / 4 DVE by default):
  - PE taps run as PSUM-accumulated matmuls with diagonal bf16 lhsT weights
    against shifted views of the padded tile;
  - DVE taps run as tensor_scalar at 4x mode + tensor_tensor add at 2x mode
    over 32-row blocks and are injected into the PSUM accumulation via
    identity matmuls.
Leaky-relu runs on the Scalar engine (Prelu, PSUM->SBUF, bf16); the final 1x1
conv is one matmul per 512-column chunk with a block-diagonal
[conv_w.T, conv_w.T] lhsT; conv bias rides the Scalar engine's Identity
activation during the PSUM->SBUF copy (one op per 1024 columns); outputs
leave as bf16 and are widened to fp32 on the host.  The tiny channel-
attention/kernel-generating MLPs run once per core on the PE/ACT engines.
"""

import sys

if "/opt/trn_rl_repo" not in sys.path:
    sys.path.append("/opt/trn_rl_repo")

import numpy as np
import ml_dtypes

B, C, H, W = 32, 64, 128, 128
KK = 3
RED = 8
N_CORES = 8
BPC = B // N_CORES          # samples per core (4)
PAIRS = BPC // 2            # sample pairs per core (2)
HP, WP = H + 2, W + 2       # zero-padded image dims (130)
ROWS_PER_CHUNK = 4          # output rows per matmul chunk -> N = 4*128 = 512
NCHUNK = H // ROWS_PER_CHUNK

_CACHE = {}

ALL_TAPS = [(di, dj) for di in range(KK) for dj in range(KK)]

WPP = 132  # host-padded image width (zero cols 0, 129..131); rows 0/129 zero

CONFIGS = {
    # v5: v3 + software-pipelined prep: the MLP/gather/diag chain for the
    # next main-loop pass is issued ahead of the current pass (2-unrolled
    # repeat body with explicit slot tags), hiding its cross-engine latency.
    "v5": dict(pe_taps=5, grp=4, big_psb=False, hoist=True, db=False,
               hbmpad=False, skew=True),
    "v5pad": dict(pe_taps=5, grp=4, big_psb=False, hoist=False, db=False,
                  hbmpad=True, skew=True),
    # v4: v3 + host-side zero-padded input in HBM -> single fully-contiguous
    # SBUF destination per x tile (4KB+ DMA descriptor runs instead of 256B,
    # which halves effective DMA bandwidth), no border memsets needed.
    "v4": dict(pe_taps=5, grp=4, big_psb=False, hoist=False, db=True,
               hbmpad=True),
    "v4p6": dict(pe_taps=6, grp=4, big_psb=False, hoist=False, db=True,
                 hbmpad=True),
    # v3: GRP=4 + hoisted border memsets + double-buffered MLP/diag stage
    # pools so iteration k+1's prep overlaps iteration k's main loop.
    "v3": dict(pe_taps=5, grp=4, big_psb=False, hoist=True, db=True),
    "v3p6": dict(pe_taps=6, grp=4, big_psb=False, hoist=True, db=True),
    # v2: GRP=2 psum groups, FD=1024 evacs on both ACT passes, hoisted
    # border memsets.
    "v2": dict(pe_taps=5, grp=2, big_psb=True, hoist=True),
    "v2p6": dict(pe_taps=6, grp=2, big_psb=True, hoist=True),
    "v2p4": dict(pe_taps=4, grp=2, big_psb=True, hoist=True),
    # v1: legacy structure (GRP=4, FD=512 bias evac, per-pair memsets)
    "p5": dict(pe_taps=5, grp=4, big_psb=False, hoist=False),
    "p6": dict(pe_taps=6, grp=4, big_psb=False, hoist=False),
    "p4": dict(pe_taps=4, grp=4, big_psb=False, hoist=False),
}


def _tap_split(pe_taps):
    # dj==1 taps are misaligned for DVE 2x/4x modes (bf16), keep them on PE.
    if pe_taps == 6:
        dve = [(0, 0), (0, 2), (1, 0)]
        pe = [(0, 1), (1, 1), (2, 1), (2, 0), (2, 2), (1, 2)]
    elif pe_taps == 4:
        dve = [(0, 0), (0, 2), (1, 0), (1, 2), (2, 0)]
        pe = [(0, 1), (1, 1), (2, 1), (2, 2)]
    else:
        dve = [(0, 0), (0, 2), (1, 0), (1, 2)]
        pe = [(0, 1), (1, 1), (2, 1), (2, 0), (2, 2)]
    return pe, dve


def _build(repeat=1, cfg="v2"):
    import concourse.bass as bass  # noqa: F401
    from concourse import bacc, tile, mybir

    cc = CONFIGS[cfg]
    f32 = mybir.dt.float32
    bf16 = mybir.dt.bfloat16
    AF = mybir.ActivationFunctionType

    nc = bacc.Bacc(None, target_bir_lowering=False, debug=False)

    if cc.get("hbmpad"):
        x = nc.dram_tensor("xp", [BPC, C, HP, WPP], bf16,
                           kind="ExternalInput")
    else:
        x = nc.dram_tensor("x", [BPC, C, H, W], bf16, kind="ExternalInput")
    vt = nc.dram_tensor("vt", [C, BPC], f32, kind="ExternalInput")
    caw1t = nc.dram_tensor("caw1t", [C, RED], f32, kind="ExternalInput")
    caw2t = nc.dram_tensor("caw2t", [RED, C], f32, kind="ExternalInput")
    kw1t = nc.dram_tensor("kw1t", [C, C], f32, kind="ExternalInput")
    kw2t = nc.dram_tensor("kw2t", [C, C * KK * KK], f32, kind="ExternalInput")
    convt = nc.dram_tensor("convt", [128, 128], bf16, kind="ExternalInput")
    bcol = nc.dram_tensor("bcol", [128, 1], f32, kind="ExternalInput")
    eye = nc.dram_tensor("eye", [128, 128], f32, kind="ExternalInput")
    eyebf = nc.dram_tensor("eyebf", [128, 128], bf16, kind="ExternalInput")
    out = nc.dram_tensor("out", [BPC, C, H, W], bf16, kind="ExternalOutput")

    NK = C * KK * KK  # 576
    psa_bufs = 2 if cc["grp"] == 2 else 3
    stage_bufs = 2 if cc.get("db") else 1

    with tile.TileContext(nc) as tc:
        with (
            tc.tile_pool(name="consts", bufs=1) as consts,
            tc.tile_pool(name="stage", bufs=stage_bufs) as stage,
            tc.tile_pool(name="diags", bufs=stage_bufs) as diags,
            tc.tile_pool(name="xin", bufs=2) as xin,
            tc.tile_pool(name="parts", bufs=2) as parts,
            tc.tile_pool(name="ys", bufs=5) as ys,
            tc.tile_pool(name="os", bufs=6) as osb,
            tc.tile_pool(name="psA", bufs=psa_bufs, space="PSUM") as psA,
            tc.tile_pool(name="psB", bufs=2, space="PSUM") as psB,
        ):
            # ---- constants into SBUF ----
            vt_sb = consts.tile([C, BPC], f32)
            caw1t_sb = consts.tile([C, RED], f32)
            caw2t_sb = consts.tile([RED, C], f32)
            kw1t_sb = consts.tile([C, C], f32)
            kw2t_sb = consts.tile([C, NK], f32)
            convt_sb = consts.tile([128, 128], bf16)
            bcol_sb = consts.tile([128, 1], f32)
            eye_sb = consts.tile([128, 128], f32)
            eyebf_sb = consts.tile([128, 128], bf16)
            for t, d in (
                (vt_sb, vt), (caw1t_sb, caw1t), (caw2t_sb, caw2t),
                (kw1t_sb, kw1t), (kw2t_sb, kw2t), (convt_sb, convt),
                (bcol_sb, bcol), (eye_sb, eye), (eyebf_sb, eyebf),
            ):
                nc.sync.dma_start(out=t[:], in_=d.ap())

            xts = None
            if cc["hoist"]:
                # Pre-zero the one-pixel border of both rotating x tiles
                # once; the interior is fully overwritten every pair and
                # the border stays zero across repeat iterations.
                xts = [xin.tile([128, HP, WP], bf16, tag="xt",
                                name=f"xt_pre{i}")
                       for i in range(2)]
                for xt in xts:
                    nc.vector.memset(xt[:, 0, :], 0.0)
                    nc.vector.memset(xt[:, HP - 1, :], 0.0)
                    nc.vector.memset(xt[:, 1:HP - 1, 0], 0.0)
                    nc.vector.memset(xt[:, 1:HP - 1, WP - 1], 0.0)

            args = (nc, tc, mybir, AF, f32, bf16, cc,
                    consts, stage, diags, xin, parts, ys, osb,
                    psA, psB,
                    vt_sb, caw1t_sb, caw2t_sb, kw1t_sb, kw2t_sb,
                    convt_sb, bcol_sb, eye_sb, eyebf_sb, x, out)
            import contextlib
            if cc.get("skew"):
                # All preps at the body head: each main's diag/dcols chain
                # is issued U-1 mains ahead of its consumer, so only main0
                # pays the prep-chain latency once per unrolled body.
                U = 4
                if repeat == 1:
                    ctx0 = _prep(*args, slot=0)
                    _main(*args, prep=ctx0)
                else:
                    assert repeat % U == 0, "skew configs need repeat % 4 == 0"
                    with tc.For_i(0, repeat // U, 1):
                        ctxs = [_prep(*args, slot=s) for s in range(U)]
                        for s in range(U):
                            _main(*args, prep=ctxs[s])
            else:
                rep_ctx = (tc.For_i(0, repeat, 1) if repeat > 1
                           else contextlib.nullcontext())
                with rep_ctx:
                    ctx = _prep(*args, slot=0)
                    _main(*args, prep=ctx)

    nc.compile()
    return nc


def _prep(nc, tc, mybir, AF, f32, bf16, cc,
          consts, stage, diags, xin, parts, ys, osb, psA, psB,
          vt_sb, caw1t_sb, caw2t_sb, kw1t_sb, kw2t_sb,
          convt_sb, bcol_sb, eye_sb, eyebf_sb, x, out, slot=0):
    NK = C * KK * KK
    PE_TAPS, DVE_TAPS = _tap_split(cc["pe_taps"])
    sl = f"_{slot}"

    # ---- tiny MLP stage: attention + generated kernels ----
    # h1T = lrelu(ca_w1 @ v.T)                       [8, BPC]
    ps_h1 = psA.tile([RED, BPC], f32, tag="pa", name=f"ps_h1{sl}")
    nc.tensor.matmul(ps_h1[:], lhsT=caw1t_sb[:], rhs=vt_sb[:],
                     start=True, stop=True)
    h1t_sb = stage.tile([RED, BPC], f32, tag=f"h1t{sl}", name=f"h1t{sl}")
    nc.scalar.activation(h1t_sb[:], ps_h1[:], AF.Prelu, alpha=0.1)

    # attT = sigmoid(ca_w2 @ h1)                     [64, BPC]
    ps_att = psA.tile([C, BPC], f32, tag="pa", name=f"ps_att{sl}")
    nc.tensor.matmul(ps_att[:], lhsT=caw2t_sb[:], rhs=h1t_sb[:],
                     start=True, stop=True)
    att_sb = stage.tile([C, BPC], f32, tag=f"att{sl}", name=f"att{sl}")
    nc.scalar.activation(att_sb[:], ps_att[:], AF.Sigmoid)

    # h2T = lrelu(k_w1 @ v.T)                        [64, BPC]
    ps_h2 = psA.tile([C, BPC], f32, tag="pa", name=f"ps_h2{sl}")
    nc.tensor.matmul(ps_h2[:], lhsT=kw1t_sb[:], rhs=vt_sb[:],
                     start=True, stop=True)
    h2t_sb = stage.tile([C, BPC], f32, tag=f"h2t{sl}", name=f"h2t{sl}")
    nc.scalar.activation(h2t_sb[:], ps_h2[:], AF.Prelu, alpha=0.1)

    # kern = h2 @ k_w2.T                             [BPC, 576]
    ps_k = psA.tile([BPC, NK], f32, tag="pa", name=f"ps_k{sl}")
    nc.tensor.matmul(ps_k[:, 0:512], lhsT=h2t_sb[:],
                     rhs=kw2t_sb[:, 0:512], start=True, stop=True)
    nc.tensor.matmul(ps_k[:, 512:NK], lhsT=h2t_sb[:],
                     rhs=kw2t_sb[:, 512:NK], start=True, stop=True)
    kern_sb = stage.tile([BPC, NK], f32, tag=f"kern{sl}", name=f"kern{sl}")
    nc.scalar.activation(kern_sb[:], ps_k[:], AF.Copy)

    # ---- gather per-pair tap scalars: p = s*64 + c on partitions ----
    dtap_sb = stage.tile([128, PAIRS, KK * KK], f32, tag=f"dtap{sl}",
                         name=f"dtap{sl}")
    attpp_sb = stage.tile([128, PAIRS], f32, tag=f"attpp{sl}",
                          name=f"attpp{sl}")
    for pr in range(PAIRS):
        for s in range(2):
            b = pr * 2 + s
            # src [1, 64, 9] and dest [64, 1, 9] match in flattened
            # element order (dma_start maps by flat AP order).
            src = kern_sb[b:b + 1, :].rearrange(
                "o (c t) -> o c t", c=C)
            nc.sync.dma_start(
                out=dtap_sb[C * s:C * (s + 1), pr:pr + 1, :], in_=src)
            nc.sync.dma_start(
                out=attpp_sb[C * s:C * (s + 1), pr:pr + 1],
                in_=att_sb[:, b:b + 1])

    # d[p, t] = att[p] * kern[p, t]; diag tiles = eye * d[:, t]
    diag = [{} for _ in range(PAIRS)]
    dcols = []
    for pr in range(PAIRS):
        d_pr = stage.tile([128, KK * KK], f32, tag=f"d{slot}_{pr}",
                          name=f"d{slot}_{pr}")
        nc.vector.tensor_scalar_mul(
            d_pr[:], dtap_sb[:, pr, :], attpp_sb[:, pr:pr + 1])
        dcols.append(d_pr)
        for (di, dj) in PE_TAPS:
            t = di * KK + dj
            dg = diags.tile([128, 128], bf16, tag=f"diag{slot}_{pr}_{t}",
                            name=f"diag{slot}_{pr}_{t}")
            nc.vector.tensor_scalar_mul(
                dg[:], eye_sb[:], d_pr[:, t:t + 1])
            diag[pr][(di, dj)] = dg
    return {"diag": diag, "dcols": dcols}


def _main(nc, tc, mybir, AF, f32, bf16, cc,
          consts, stage, diags, xin, parts, ys, osb, psA, psB,
          vt_sb, caw1t_sb, caw2t_sb, kw1t_sb, kw2t_sb,
          convt_sb, bcol_sb, eye_sb, eyebf_sb, x, out, prep=None):
    PE_TAPS, DVE_TAPS = _tap_split(cc["pe_taps"])
    GRP = cc["grp"]
    diag = prep["diag"]
    dcols = prep["dcols"]
    _main.calls = getattr(_main, "calls", 0) + 1
    mn = f"m{_main.calls}"

    # ---- main loop ----
    xv = x.ap().rearrange("(pr s) c h w -> pr (s c) h w", pr=PAIRS)
    ov = out.ap().rearrange("(pr s) c h w -> pr (s c) h w", pr=PAIRS)

    NW = ROWS_PER_CHUNK * W  # 512
    for pr in range(PAIRS):
        if cc.get("hbmpad"):
            # input is zero-padded in HBM; each row-slice destination is
            # fully contiguous in SBUF (>= 4KB descriptor runs).
            xt = xin.tile([128, HP, WPP], bf16, tag="xt")
            nsplit = 8
            bounds = [round(HP * k / nsplit) for k in range(nsplit + 1)]
            for k in range(nsplit):
                a, b = bounds[k], bounds[k + 1]
                nc.sync.dma_start(
                    out=xt[:, a:b, :], in_=xv[pr, :, a:b, :])
        else:
            xt = xin.tile([128, HP, WP], bf16, tag="xt")
            if not cc["hoist"]:
                nc.vector.memset(xt[:, 0, :], 0.0)
                nc.vector.memset(xt[:, HP - 1, :], 0.0)
                nc.vector.memset(xt[:, 1:HP - 1, 0], 0.0)
                nc.vector.memset(xt[:, 1:HP - 1, WP - 1], 0.0)
            # split the 4 MiB load across DMA queues
            nsplit = 8
            rstep = H // nsplit
            for k in range(nsplit):
                r0 = k * rstep
                nc.sync.dma_start(
                    out=xt[:, 1 + r0:1 + r0 + rstep, 1:WP - 1],
                    in_=xv[pr, :, r0:r0 + rstep, :])

        # Vector engine: even-aligned taps per 32-row block (bf16 2x/4x)
        BR = 32
        parts_of = {}
        for gb in range(0, NCHUNK, BR // ROWS_PER_CHUNK):
            r0 = (gb // (BR // ROWS_PER_CHUNK)) * BR
            part = parts.tile([128, BR, W], bf16, tag="part",
                              bufs=3, name=f"part{gb}")
            for g2 in range(gb, gb + BR // ROWS_PER_CHUNK):
                parts_of[g2] = (part, (g2 - gb) * ROWS_PER_CHUNK)
            for n, (di, dj) in enumerate(DVE_TAPS):
                t = di * KK + dj
                xin_v = xt[:, r0 + di:r0 + di + BR, dj:dj + W]
                if n == 0:
                    nc.vector.tensor_scalar_mul(
                        part[:], xin_v, dcols[pr][:, t:t + 1])
                else:
                    tmp = parts.tile([128, BR, W], bf16, tag="tmp",
                                     bufs=3, name=f"tmp{gb}_{n}")
                    nc.vector.tensor_scalar_mul(
                        tmp[:], xin_v, dcols[pr][:, t:t + 1])
                    nc.vector.tensor_add(part[:], part[:], tmp[:])

        if GRP == 2:
            for g in range(0, NCHUNK, 2):
                part, roff = parts_of[g]
                pa = psA.tile([128, 2 * NW], f32, tag="pa",
                              name=f"pa{pr}_{g}")
                for t, (di, dj) in enumerate(PE_TAPS):
                    for c in range(2):
                        i0 = (g + c) * ROWS_PER_CHUNK
                        nc.tensor.matmul(
                            pa[:, c * NW:c * NW + NW],
                            lhsT=diag[pr][(di, dj)][:],
                            rhs=xt[:, i0 + di:i0 + di + ROWS_PER_CHUNK,
                                   dj:dj + W],
                            start=(t == 0), stop=False,
                            skip_group_check=True)
                for c in range(2):
                    rc = roff + c * ROWS_PER_CHUNK
                    nc.tensor.matmul(
                        pa[:, c * NW:c * NW + NW],
                        lhsT=eyebf_sb[:],
                        rhs=part[:, rc:rc + ROWS_PER_CHUNK, :],
                        start=False, stop=True, skip_group_check=True)
                yt = ys.tile([128, 2 * NW], bf16, tag="yt")
                nc.scalar.activation(yt[:], pa[:], AF.Prelu, alpha=0.1)
                pb = psB.tile([128, 2 * NW], f32, tag="pb")
                for c2 in range(2):
                    nc.tensor.matmul(
                        pb[:, c2 * NW:c2 * NW + NW], lhsT=convt_sb[:],
                        rhs=yt[:, c2 * NW:c2 * NW + NW],
                        start=True, stop=True)
                ot = osb.tile([128, 2 * NW], bf16, tag="ot")
                nc.scalar.activation(ot[:], pb[:], AF.Identity,
                                     bias=bcol_sb[:, 0:1])
                j0 = g * ROWS_PER_CHUNK
                nc.sync.dma_start(
                    out=ov[pr, :, j0:j0 + 2 * ROWS_PER_CHUNK, :],
                    in_=ot[:].rearrange("p (r w) -> p r w",
                                        r=2 * ROWS_PER_CHUNK))
        else:
            for g in range(0, NCHUNK, GRP):
                part, roff = parts_of[g]
                pas = [psA.tile([128, 2 * NW], f32,
                                tag="pa", name=f"pa{g}_{h}")
                       for h in range(GRP // 2)]
                for t, (di, dj) in enumerate(PE_TAPS):
                    for c in range(GRP):
                        i0 = (g + c) * ROWS_PER_CHUNK
                        nc.tensor.matmul(
                            pas[c // 2][:, (c % 2) * NW:
                                        (c % 2) * NW + NW],
                            lhsT=diag[pr][(di, dj)][:],
                            rhs=xt[:, i0 + di:i0 + di + ROWS_PER_CHUNK,
                                   dj:dj + W],
                            start=(t == 0), stop=False,
                            skip_group_check=True)
                for c in range(GRP):
                    rc = roff + c * ROWS_PER_CHUNK
                    nc.tensor.matmul(
                        pas[c // 2][:, (c % 2) * NW:(c % 2) * NW + NW],
                        lhsT=eyebf_sb[:],
                        rhs=part[:, rc:rc + ROWS_PER_CHUNK, :],
                        start=False, stop=True, skip_group_check=True)
                for h in range(GRP // 2):
                    i0 = (g + 2 * h) * ROWS_PER_CHUNK
                    yt = ys.tile([128, 2 * NW], bf16, tag="yt")
                    nc.scalar.activation(yt[:], pas[h][:], AF.Prelu,
                                         alpha=0.1)
                    for c2 in range(2):
                        pb = psB.tile([128, NW], f32, tag="pb")
                        nc.tensor.matmul(
                            pb[:], lhsT=convt_sb[:],
                            rhs=yt[:, c2 * NW:c2 * NW + NW],
                            start=True, stop=True)
                        ot = osb.tile([128, NW], bf16, tag="ot")
                        nc.scalar.activation(ot[:], pb[:], AF.Identity,
                                             bias=bcol_sb[:, 0:1])
                        j0 = i0 + c2 * ROWS_PER_CHUNK
                        nc.sync.dma_start(
                            out=ov[pr, :, j0:j0 + ROWS_PER_CHUNK, :],
                            in_=ot[:].rearrange("p (r w) -> p r w",
                                                r=ROWS_PER_CHUNK))


def get_nc(repeat=1, cfg="v5"):
    key = ("nc", repeat, cfg)
    if key not in _CACHE:
        _CACHE[key] = _build(repeat, cfg)
    return _CACHE[key]


def make_in_maps(x0, v, ca_w1, ca_w2, k_w1, k_w2, conv_w, conv_b):
    bf = ml_dtypes.bfloat16
    caw1t = np.ascontiguousarray(ca_w1.T, dtype=np.float32)
    caw2t = np.ascontiguousarray(ca_w2.T, dtype=np.float32)
    kw1t = np.ascontiguousarray(k_w1.T, dtype=np.float32)
    kw2t = np.ascontiguousarray(k_w2.T, dtype=np.float32)
    convt = np.zeros((128, 128), dtype=bf)
    cwt = conv_w.T.astype(bf)
    convt[0:64, 0:64] = cwt
    convt[64:128, 64:128] = cwt
    bcol = np.tile(conv_b.astype(np.float32), 2)[:, None].copy()
    eye = np.eye(128, dtype=np.float32)
    eyebf = np.eye(128, dtype=bf)
    x0bf = x0.astype(bf)
    xpad = np.zeros((B, C, HP, WPP), dtype=bf)
    xpad[:, :, 1:H + 1, 1:W + 1] = x0bf
    in_maps = []
    for k in range(N_CORES):
        sl = slice(k * BPC, (k + 1) * BPC)
        in_maps.append({
            "x": np.ascontiguousarray(x0bf[sl]),
            "xp": np.ascontiguousarray(xpad[sl]),
            "vt": np.ascontiguousarray(v[sl].T, dtype=np.float32),
            "caw1t": caw1t, "caw2t": caw2t, "kw1t": kw1t, "kw2t": kw2t,
            "convt": convt, "bcol": bcol, "eye": eye, "eyebf": eyebf,
        })
    return in_maps


def kernel(x0, v, ca_w1, ca_w2, k_w1, k_w2, conv_w, conv_b):
    from concourse.bass_utils import run_bass_kernel_spmd

    nc = get_nc()
    in_maps = make_in_maps(x0, v, ca_w1, ca_w2, k_w1, k_w2, conv_w, conv_b)
    res = run_bass_kernel_spmd(nc, in_maps, list(range(N_CORES)))
    return np.concatenate([res.results[i]["out"] for i in range(N_CORES)],
                          axis=0).astype(np.float32)
